# revision 16
# baseline (speedup 1.0000x reference)
"""Trainium2 Bass kernel: batched pairwise Hessian blocks (Coords2Stress).

For each example b:  out[b, 3i+a, 3j+c] = -sep_a*sep_c/(|sep|^2+eps) for the
off-diagonal atom blocks (masked to the valid atom count), with the 3x3
diagonal blocks overwritten by the negative row-sums.

v2: block-upper-triangle + engine-balanced.

The 3x3 blocks are symmetric in (a,c) AND block(i,j) == block(j,i), so the
device only computes the block upper triangle: item (b,t) covers rows
[128t,128t+128) x cols [128t, max(na,128(t+1))) in item-local column coords
j' = j - 128t.  The host mirrors each rectangle (plain transpose) into the
lower triangle.  Per-slot engine assignment:
  * GpSimd broadcasts the column-coordinate planes cfp[1,3w] -> cb[128,3w].
  * ScalarE (Act) builds s~_c = cb_c - ct_c (Identity with per-partition
    bias), and casts the finished row tile to fp16 for the column sums.
  * TensorE computes the negated masked d2 via the |x|^2+|y|^2-2xy expansion
    (k=5 matmul; validity masks folded in as +BIG so the reciprocal
    vanishes), and column sums of the fp16 row tile (ones-lhsT matmuls,
    3 chunks stacked per PSUM bank at base partitions 0/32/64).
  * VectorE: reciprocal_approx_fast(r0 = 1/-d2), g~_a = s~_a * r0, and the
    9 products row[p,a,j,c] = g~_a * s~_c (6 unique carry dac accumulation).
Diagonal blocks: diag(q) = -(dac_q + sum of exported column sums from items
strictly above q's tile); the host adds the exported pieces and writes the
blocks.  Unwritten output stays zero.
"""

import os
import sys

import numpy as np

for _p in ("/opt/trn_rl_repo", "/root/.axon_site/_ro/trn_rl_repo"):
    if os.path.isdir(_p) and _p not in sys.path:
        sys.path.insert(0, _p)

import concourse.bass as bass
import concourse.bacc as bacc
import concourse.tile as tile
from concourse import mybir
from concourse.bass_utils import run_bass_kernel_spmd

N_CORES = 8
P = 128  # atoms per work item == SBUF partitions
CH = 512  # matmul free-dim chunk (one PSUM bank of fp32)
QPB = 3  # colsum chunks stacked per PSUM bank (base partitions 0/32/64)
EPS = 1e-5
BIG = 1e30
F32 = mybir.dt.float32
F16 = mybir.dt.float16
OP = mybir.AluOpType
UNIQ = [(0, 0), (0, 1), (0, 2), (1, 1), (1, 2), (2, 2)]
MIRROR = [(1, 0), (2, 0), (2, 1)]
SYM6 = np.array([[0, 1, 2], [1, 3, 4], [2, 4, 5]])


def _plan(num_atoms):
    """Triangle work items -> slots.  Item (b,t) covers local columns
    [0, wt) with wt = max(na, 128(t+1)) - 128t.  Returns
    [(slot_width, [(wt, b, t), ...])] with slots of <= N_CORES items."""
    items = []
    for b, na in enumerate(num_atoms):
        na = int(na)
        if na <= 0:
            continue
        nt = -(-na // P)
        for t in range(nt):
            items.append((max(na, P * (t + 1)) - P * t, b, t))
    items.sort(key=lambda x: (-x[0], x[1], x[2]))
    nslot = max(1, -(-len(items) // N_CORES))
    slots = []
    for k in range(nslot):
        chunk = items[k * N_CORES:(k + 1) * N_CORES]
        slots.append((chunk[0][0], chunk))
    return slots


CPG = 6  # colsum chunks per PSUM group tile ([P, 2*CH] = 2 banks, 3 quadrants)


def _ncs(w):
    """Number of colsum chunks for slot width w."""
    return -(-9 * w // CH)


def _nblk(w):
    """Number of colsum PSUM group tiles (CPG chunks each)."""
    return -(-_ncs(w) // CPG)


def _offsets(widths):
    rh, cf, oo, cs = [], [], [], []
    a = b = c = d = 0
    for w in widths:
        rh.append(a)
        cf.append(b)
        oo.append(c)
        cs.append(d)
        a += w
        b += 3 * w
        c += 384 * 3 * w
        d += QPB * _nblk(w) * 2 * CH
    return rh, cf, oo, cs, a, b, c, d


def _chunks(w):
    return [(h, min(h + CH, w)) for h in range(0, w, CH)]


def _build(widths):
    """Emit + compile the SPMD program for the given per-slot widths."""
    K = len(widths)
    Wmax = max(widths)
    rh_off, cf_off, out_off, cs_off, rh_len, cf_len, out_len, cs_len = (
        _offsets(widths))
    AUXW = P * K + 3 * K

    nc = bacc.Bacc("TRN2", target_bir_lowering=False, debug=False)
    d_rhs = nc.dram_tensor("rhs", [5, rh_len], F32, kind="ExternalInput").ap()
    d_cfp = nc.dram_tensor("cfp", [1, cf_len], F32, kind="ExternalInput").ap()
    d_aux = nc.dram_tensor("aux", [P, AUXW], F32, kind="ExternalInput").ap()
    d_out = nc.dram_tensor("out", [out_len], F32, kind="ExternalOutput").ap()
    d_dg = nc.dram_tensor("dg", [K, P, 6], F32, kind="ExternalOutput").ap()
    d_cs = nc.dram_tensor("cs", [cs_len], F32, kind="ExternalOutput").ap()

    with tile.TileContext(nc) as tc:
        with (
            tc.tile_pool(name="const", bufs=1) as constp,
            tc.tile_pool(name="rhsp", bufs=2) as rhsp,
            tc.tile_pool(name="cfpp", bufs=1) as cfpp,
            tc.tile_pool(name="sp", bufs=2) as sp,
            tc.tile_pool(name="rp", bufs=2) as rp,
            tc.tile_pool(name="gp", bufs=2) as gp,
            tc.tile_pool(name="dac", bufs=2) as dacp,
            tc.tile_pool(name="row", bufs=2) as rowp,
            tc.tile_pool(name="rbp", bufs=2) as rbp,
            tc.tile_pool(name="csp", bufs=2) as csbp,
            tc.psum_pool(name="d2p", bufs=2) as d2pp,
            tc.psum_pool(name="csm", bufs=2) as csmp,
        ):
            aux = constp.tile([P, AUXW], F32)
            nc.scalar.dma_start(out=aux[:], in_=d_aux)
            onesh = constp.tile([P, 1], F16)
            nc.vector.memset(onesh[:], 1.0)

            for k, w in enumerate(widths):
                lhsT = aux[0:5, P * k: P * k + P]
                nblk = _nblk(w)

                rhs = rhsp.tile([5, Wmax], F32, tag="rhs")
                nc.sync.dma_start(
                    out=rhs[:, :w], in_=d_rhs[:, rh_off[k]: rh_off[k] + w])
                cfp = cfpp.tile([1, 3 * Wmax], F32, tag="cfp")
                nc.sync.dma_start(
                    out=cfp[:, :3 * w],
                    in_=d_cfp[:, cf_off[k]: cf_off[k] + 3 * w])

                # s~_c = cf_c - ct_c: gpsimd broadcasts the cf plane into the
                # s tile and adds the per-partition bias -ct_c in place
                s_pl = []
                for c in range(3):
                    s_c = sp.tile([P, Wmax], F32, tag=f"s{c}")
                    nc.gpsimd.partition_broadcast(
                        s_c[:, :w], cfp[:1, c * w: c * w + w])
                    nc.gpsimd.tensor_scalar_add(
                        s_c[:, :w], s_c[:, :w],
                        aux[:, P * K + 3 * k + c: P * K + 3 * k + c + 1])
                    s_pl.append(s_c)

                # negated masked d2 via TensorE (stays in PSUM for the recip)
                d2 = d2pp.tile([P, Wmax], F32, tag="d2")
                for (h0, h1) in _chunks(w):
                    nc.tensor.matmul(
                        d2[:, h0:h1], lhsT, rhs[:, h0:h1])

                # r0 = 1/(-d2)  (negative; masks make it ~0 where invalid)
                r0 = rp.tile([P, Wmax], F32, tag="r0")
                nc.vector.reciprocal_approx_fast(r0[:, :w], d2[:, :w])

                # g~_a = s~_a * r0
                g_pl = []
                for a in range(3):
                    g_a = gp.tile([P, Wmax], F32, tag=f"g{a}")
                    nc.vector.tensor_tensor(g_a[:, :w], s_pl[a][:, :w],
                                            r0[:, :w], OP.mult)
                    g_pl.append(g_a)

                # row[p, a, j, c] = g~_a * s~_c = -sep_a*sep_c*m/d2
                row = rowp.tile([P, 9 * Wmax], F32, tag="row")
                row4 = row[:, :9 * w].rearrange("p (a j c) -> p a j c",
                                                a=3, c=3)
                dac = dacp.tile([P, 8], F32, tag="dac")
                for i, (a, c) in enumerate(UNIQ):
                    nc.vector.scalar_tensor_tensor(
                        row4[:, a, :, c], s_pl[c][:, :w], 0.0, g_pl[a][:, :w],
                        OP.bypass, OP.mult,
                        accum_out=dac[:, i: i + 1])
                for (a, c) in MIRROR:
                    nc.vector.scalar_tensor_tensor(
                        row4[:, a, :, c], s_pl[c][:, :w], 0.0, g_pl[a][:, :w],
                        OP.bypass, OP.mult)
                nc.sync.dma_start(out=d_dg[k], in_=dac[:, 0:6])

                # column sums: fp16 cast (Act) + ones-matmuls; CPG chunks per
                # PSUM group tile ([P, 2CH], quadrants 0/32/64 x 2 segments),
                # one Act drain per group, 3 export DMAs per slot
                csb = csbp.tile([P, nblk * 2 * CH], F32, tag="csb")
                for blk in range(nblk):
                    c0 = blk * CPG * CH
                    c1 = min(c0 + CPG * CH, 9 * w)
                    rb = rbp.tile([P, CPG * CH], F16, tag="rb")
                    nc.scalar.copy(rb[:, :c1 - c0], row[:, c0:c1])
                    ps = csmp.tile([P, 2 * CH], F32, tag="cs")
                    for j in range(CPG):
                        q0 = j * CH
                        q1 = min(q0 + CH, c1 - c0)
                        if q0 >= c1 - c0:
                            break
                        q, seg = j % QPB, j // QPB
                        nc.tensor.matmul(
                            ps[32 * q: 32 * q + 1,
                               seg * CH: seg * CH + q1 - q0], onesh[:],
                            rb[:, q0:q1])
                    nc.scalar.copy(csb[:, blk * 2 * CH:(blk + 1) * 2 * CH],
                                   ps[:])
                for q in range(QPB):
                    # row 32q of csb holds chunks with (c % CPG) % QPB == q
                    nc.gpsimd.dma_start(
                        out=d_cs[cs_off[k] + q * nblk * 2 * CH:
                                 cs_off[k] + (q + 1) * nblk * 2 * CH]
                        .unsqueeze(0),
                        in_=csb[32 * q: 32 * q + 1, :])

                dro = (d_out[out_off[k]: out_off[k] + 384 * 3 * w]
                       .rearrange("(p a n) -> p a n", p=P, a=3))
                nc.sync.dma_start(
                    out=dro,
                    in_=row[:, :9 * w].rearrange("p (a n) -> p a n", a=3))
    nc.compile()
    return nc


def _pack(coords, num_atoms, slots):
    """Per-core input arrays for the SPMD program."""
    B = coords.shape[0]
    N = coords.shape[1] // 3
    widths = [s[0] for s in slots]
    K = len(slots)
    AUXW = P * K + 3 * K
    rh_off, cf_off, out_off, cs_off, rh_len, cf_len, out_len, cs_len = (
        _offsets(widths))
    c3 = coords.reshape(B, N, 3)
    pidx = np.arange(P)

    in_maps = []
    for _ in range(N_CORES):
        in_maps.append({
            "rhs": np.zeros((5, rh_len), np.float32),
            "cfp": np.zeros((1, cf_len), np.float32),
            "aux": np.zeros((P, AUXW), np.float32),
        })

    placement = []  # (core, k, b, t, wt)
    for k, (w, chunk) in enumerate(slots):
        for core, (wt, b, t) in enumerate(chunk):
            placement.append((core, k, b, t, wt))
            m = in_maps[core]
            na = int(num_atoms[b])
            j0 = t * P
            cf = c3[b, j0: j0 + w].astype(np.float64)   # [<=w, 3] local cols
            nw = cf.shape[0]
            ct = c3[b, j0: j0 + P].astype(np.float64)   # [<=P, 3] own tile
            np_ = ct.shape[0]
            colmask = (j0 + np.arange(nw)) < na
            rowvalid = (j0 + pidx) < na
            # d2 rhs block: out = -(q_p + |cf|^2+eps+BIG*~m - 2 ct.cf)
            o = rh_off[k]
            rr = m["rhs"]
            rr[0, o: o + w] = -1.0
            rr[1, o: o + nw] = -((cf * cf).sum(1) + EPS + BIG * (~colmask))
            rr[1, o + nw: o + w] = -BIG
            for c in range(3):
                rr[2 + c, o: o + nw] = 2.0 * cf[:, c]
            # coordinate planes for the broadcast (+cf, c-major)
            for c in range(3):
                m["cfp"][0, cf_off[k] + c * w: cf_off[k] + c * w + nw] = (
                    cf[:, c])
            # lhsT block rows: q, ones, ct_x, ct_y, ct_z
            a0 = P * k
            q = np.full(P, BIG)
            q[:np_] = (ct * ct).sum(1) + BIG * (~rowvalid[:np_])
            m["aux"][0, a0: a0 + P] = q
            m["aux"][1, a0: a0 + P] = 1.0
            for c in range(3):
                m["aux"][2 + c, a0: a0 + np_] = ct[:, c]
                # -ct_c as the per-partition bias for the s~ planes
                m["aux"][:np_, P * K + 3 * k + c] = -ct[:, c]
    return in_maps, placement


_NC_CACHE = {}


def _get_program(widths):
    key = tuple(widths)
    if key not in _NC_CACHE:
        _NC_CACHE[key] = _build(list(widths))
    return _NC_CACHE[key]


def _reassemble(results, coords_shape, num_atoms, slots, placement):
    B, threeN = coords_shape[0], coords_shape[1]
    widths = [s[0] for s in slots]
    _, _, out_off, cs_off, _, _, _, _ = _offsets(widths)
    out = np.zeros((B, threeN, threeN), np.float32)
    pidx = np.arange(P)
    a3 = np.arange(3)
    # diagonal accumulators per example/atom
    diag = [np.zeros((threeN // 3, 3, 3), np.float64) for _ in range(B)]
    for (core, k, b, t, wt) in placement:
        w = widths[k]
        res = results[core]
        blk = res["out"][out_off[k]: out_off[k] + 384 * 3 * w]
        blk = blk.reshape(384, 3 * w)
        r = 384 * t
        out[b, r:r + 384, r:r + 3 * wt] = blk[:, :3 * wt]
        # mirror the off-tile part (blocks are symmetric: plain transpose)
        if wt > P:
            out[b, r + 384: r + 3 * wt, r:r + 384] = blk[:, 384: 3 * wt].T
        # dac: row sums over the triangle window
        dg6 = res["dg"][k]                              # [P, 6]
        diag[b][t * P: t * P + P] += dg6[:, SYM6]       # [P, 3, 3]
        # column sums for atoms beyond the own tile
        if wt > P:
            nblk = _nblk(w)
            cs = res["cs"][cs_off[k]: cs_off[k] + QPB * nblk * 2 * CH]
            # export rows hold chunks c = blk*CPG + seg*QPB + q at
            # [q, blk, seg, :]; transpose to chunk order (blk, seg, q)
            cs = (cs.reshape(QPB, nblk, 2, CH).transpose(1, 2, 0, 3)
                  .reshape(-1)[:9 * w])
            cs3 = cs.reshape(3, w, 3)                   # [a, j', c]
            contrib = cs3[:, P:wt, :].transpose(1, 0, 2)  # [q', a, c]
            diag[b][t * P + P: t * P + wt] += contrib
    q3 = 3 * np.arange(threeN // 3)
    rows = q3[:, None, None] + a3[None, :, None]
    cols = q3[:, None, None] + a3[None, None, :]
    for b in range(B):
        out[b, rows, cols] = -diag[b].astype(np.float32)
    return out


LAST_RUN = None  # BassKernelResults of the most recent kernel() call


def kernel(coords, num_atoms, _trace=False):
    global LAST_RUN
    coords = np.ascontiguousarray(np.asarray(coords, dtype=np.float32))
    na = np.asarray(num_atoms).astype(np.int64)
    slots = _plan(na)
    widths = [s[0] for s in slots]
    nc = _get_program(widths)
    in_maps, placement = _pack(coords, na, slots)
    LAST_RUN = run_bass_kernel_spmd(
        nc, in_maps, list(range(N_CORES)), trace=_trace,
        tmpdir=os.environ.get("TRACE_DIR") if _trace else None)
    return _reassemble(LAST_RUN.results, coords.shape, na, slots, placement)


# revision 17
# speedup vs baseline: 1.8137x; 1.8137x over previous
"""Trainium2 Bass kernel: batched pairwise Hessian blocks (Coords2Stress).

For each example b:  out[b, 3i+a, 3j+c] = -sep_a*sep_c/(|sep|^2+eps) for the
off-diagonal atom blocks (masked to the valid atom count), with the 3x3
diagonal blocks overwritten by the negative row-sums.

v2: block-upper-triangle + engine-balanced.

The 3x3 blocks are symmetric in (a,c) AND block(i,j) == block(j,i), so the
device only computes the block upper triangle: item (b,t) covers rows
[128t,128t+128) x cols [128t, max(na,128(t+1))) in item-local column coords
j' = j - 128t.  The host mirrors each rectangle (plain transpose) into the
lower triangle.  Per-slot engine assignment:
  * GpSimd broadcasts the column-coordinate planes cfp[1,3w] -> cb[128,3w].
  * ScalarE (Act) builds s~_c = cb_c - ct_c (Identity with per-partition
    bias), and casts the finished row tile to fp16 for the column sums.
  * TensorE computes the negated masked d2 via the |x|^2+|y|^2-2xy expansion
    (k=5 matmul; validity masks folded in as +BIG so the reciprocal
    vanishes), and column sums of the fp16 row tile (ones-lhsT matmuls,
    3 chunks stacked per PSUM bank at base partitions 0/32/64).
  * VectorE: reciprocal_approx_fast(r0 = 1/-d2), g~_a = s~_a * r0, and the
    9 products row[p,a,j,c] = g~_a * s~_c (6 unique carry dac accumulation).
Diagonal blocks: diag(q) = -(dac_q + sum of exported column sums from items
strictly above q's tile); the host adds the exported pieces and writes the
blocks.  Unwritten output stays zero.
"""

import os
import sys

import numpy as np

for _p in ("/opt/trn_rl_repo", "/root/.axon_site/_ro/trn_rl_repo"):
    if os.path.isdir(_p) and _p not in sys.path:
        sys.path.insert(0, _p)

import concourse.bass as bass
import concourse.bacc as bacc
import concourse.tile as tile
from concourse import mybir
from concourse.bass_utils import run_bass_kernel_spmd

N_CORES = 8
P = 128  # atoms per work item == SBUF partitions
CH = 512  # matmul free-dim chunk (one PSUM bank of fp32)
QPB = 3  # colsum chunks stacked per PSUM bank (base partitions 0/32/64)
EPS = 1e-5
BIG = 1e30
F32 = mybir.dt.float32
F16 = mybir.dt.float16
OP = mybir.AluOpType
UNIQ = [(0, 0), (0, 1), (0, 2), (1, 1), (1, 2), (2, 2)]
MIRROR = [(1, 0), (2, 0), (2, 1)]
SYM6 = np.array([[0, 1, 2], [1, 3, 4], [2, 4, 5]])


def _plan(num_atoms):
    """Triangle work items -> slots.  Item (b,t) covers local columns
    [0, wt) with wt = max(na, 128(t+1)) - 128t.  Returns
    [(slot_width, [(wt, b, t), ...])] with slots of <= N_CORES items."""
    items = []
    for b, na in enumerate(num_atoms):
        na = int(na)
        if na <= 0:
            continue
        nt = -(-na // P)
        for t in range(nt):
            items.append((max(na, P * (t + 1)) - P * t, b, t))
    items.sort(key=lambda x: (-x[0], x[1], x[2]))
    nslot = max(1, -(-len(items) // N_CORES))
    slots = []
    for k in range(nslot):
        chunk = items[k * N_CORES:(k + 1) * N_CORES]
        slots.append((chunk[0][0], chunk))
    return slots


CPG = 6  # colsum chunks per PSUM group tile ([P, 2*CH] = 2 banks, 3 quadrants)


def _ncs(w):
    """Number of colsum chunks for slot width w."""
    return -(-9 * w // CH)


def _nblk(w):
    """Number of colsum PSUM group tiles (CPG chunks each)."""
    return -(-_ncs(w) // CPG)


def _offsets(widths):
    rh, cf, oo, cs = [], [], [], []
    a = b = c = d = 0
    for w in widths:
        rh.append(a)
        cf.append(b)
        oo.append(c)
        cs.append(d)
        a += w
        b += 3 * w
        c += 384 * 3 * w
        d += QPB * _nblk(w) * 2 * CH
    return rh, cf, oo, cs, a, b, c, d


def _chunks(w):
    return [(h, min(h + CH, w)) for h in range(0, w, CH)]


def _build(widths):
    """Emit + compile the SPMD program for the given per-slot widths."""
    K = len(widths)
    Wmax = max(widths)
    rh_off, cf_off, out_off, cs_off, rh_len, cf_len, out_len, cs_len = (
        _offsets(widths))
    AUXW = P * K + 3 * K

    nc = bacc.Bacc("TRN2", target_bir_lowering=False, debug=False)
    d_rhs = nc.dram_tensor("rhs", [5, rh_len], F32, kind="ExternalInput").ap()
    d_cfp = nc.dram_tensor("cfp", [1, cf_len], F32, kind="ExternalInput").ap()
    d_aux = nc.dram_tensor("aux", [P, AUXW], F32, kind="ExternalInput").ap()
    d_out = nc.dram_tensor("out", [out_len], F32, kind="ExternalOutput").ap()
    d_dg = nc.dram_tensor("dg", [K, P, 6], F32, kind="ExternalOutput").ap()
    d_cs = nc.dram_tensor("cs", [cs_len], F32, kind="ExternalOutput").ap()

    with tile.TileContext(nc) as tc:
        with (
            tc.tile_pool(name="const", bufs=1) as constp,
            tc.tile_pool(name="rhsp", bufs=2) as rhsp,
            tc.tile_pool(name="cfpp", bufs=1) as cfpp,
            tc.tile_pool(name="sp", bufs=2) as sp,
            tc.tile_pool(name="rp", bufs=2) as rp,
            tc.tile_pool(name="gp", bufs=2) as gp,
            tc.tile_pool(name="dac", bufs=2) as dacp,
            tc.tile_pool(name="row", bufs=2) as rowp,
            tc.tile_pool(name="rbp", bufs=2) as rbp,
            tc.tile_pool(name="csp", bufs=2) as csbp,
            tc.psum_pool(name="d2p", bufs=2) as d2pp,
            tc.psum_pool(name="csm", bufs=2) as csmp,
        ):
            aux = constp.tile([P, AUXW], F32)
            nc.scalar.dma_start(out=aux[:], in_=d_aux)
            onesh = constp.tile([P, 1], F16)
            nc.vector.memset(onesh[:], 1.0)

            for k, w in enumerate(widths):
                lhsT = aux[0:5, P * k: P * k + P]
                nblk = _nblk(w)

                rhs = rhsp.tile([5, Wmax], F32, tag="rhs")
                nc.sync.dma_start(
                    out=rhs[:, :w], in_=d_rhs[:, rh_off[k]: rh_off[k] + w])
                cfp = cfpp.tile([1, 3 * Wmax], F32, tag="cfp")
                nc.sync.dma_start(
                    out=cfp[:, :3 * w],
                    in_=d_cfp[:, cf_off[k]: cf_off[k] + 3 * w])

                # s~_c = cf_c - ct_c: gpsimd broadcasts the cf plane into the
                # s tile and adds the per-partition bias -ct_c in place
                s_pl = []
                for c in range(3):
                    s_c = sp.tile([P, Wmax], F32, tag=f"s{c}")
                    nc.gpsimd.partition_broadcast(
                        s_c[:, :w], cfp[:1, c * w: c * w + w])
                    nc.scalar.add(
                        s_c[:, :w], s_c[:, :w],
                        aux[:, P * K + 3 * k + c: P * K + 3 * k + c + 1])
                    s_pl.append(s_c)

                # negated masked d2 via TensorE (stays in PSUM for the recip)
                d2 = d2pp.tile([P, Wmax], F32, tag="d2")
                for (h0, h1) in _chunks(w):
                    nc.tensor.matmul(
                        d2[:, h0:h1], lhsT, rhs[:, h0:h1])

                # r0 = 1/(-d2)  (negative; masks make it ~0 where invalid)
                r0 = rp.tile([P, Wmax], F32, tag="r0")
                nc.vector.reciprocal_approx_fast(r0[:, :w], d2[:, :w])

                # g~_a = s~_a * r0
                g_pl = []
                for a in range(3):
                    g_a = gp.tile([P, Wmax], F32, tag=f"g{a}")
                    nc.vector.tensor_tensor(g_a[:, :w], s_pl[a][:, :w],
                                            r0[:, :w], OP.mult)
                    g_pl.append(g_a)

                # row[p, a, j, c] = g~_a * s~_c = -sep_a*sep_c*m/d2
                row = rowp.tile([P, 9 * Wmax], F32, tag="row")
                row4 = row[:, :9 * w].rearrange("p (a j c) -> p a j c",
                                                a=3, c=3)
                dac = dacp.tile([P, 8], F32, tag="dac")
                for i, (a, c) in enumerate(UNIQ):
                    nc.vector.scalar_tensor_tensor(
                        row4[:, a, :, c], s_pl[c][:, :w], 0.0, g_pl[a][:, :w],
                        OP.bypass, OP.mult,
                        accum_out=dac[:, i: i + 1])
                for (a, c) in MIRROR:
                    nc.vector.scalar_tensor_tensor(
                        row4[:, a, :, c], s_pl[c][:, :w], 0.0, g_pl[a][:, :w],
                        OP.bypass, OP.mult)
                nc.sync.dma_start(out=d_dg[k], in_=dac[:, 0:6])

                # column sums: fp16 cast (Act) + ones-matmuls; CPG chunks per
                # PSUM group tile ([P, 2CH], quadrants 0/32/64 x 2 segments),
                # one Act drain per group, 3 export DMAs per slot
                csb = csbp.tile([P, nblk * 2 * CH], F32, tag="csb")
                for blk in range(nblk):
                    c0 = blk * CPG * CH
                    c1 = min(c0 + CPG * CH, 9 * w)
                    rb = rbp.tile([P, CPG * CH], F16, tag="rb")
                    nc.scalar.copy(rb[:, :c1 - c0], row[:, c0:c1])
                    ps = csmp.tile([P, 2 * CH], F32, tag="cs")
                    for j in range(CPG):
                        q0 = j * CH
                        q1 = min(q0 + CH, c1 - c0)
                        if q0 >= c1 - c0:
                            break
                        q, seg = j % QPB, j // QPB
                        nc.tensor.matmul(
                            ps[32 * q: 32 * q + 1,
                               seg * CH: seg * CH + q1 - q0], onesh[:],
                            rb[:, q0:q1])
                    nc.scalar.copy(csb[:, blk * 2 * CH:(blk + 1) * 2 * CH],
                                   ps[:])
                for q in range(QPB):
                    # row 32q of csb holds chunks with (c % CPG) % QPB == q
                    nc.gpsimd.dma_start(
                        out=d_cs[cs_off[k] + q * nblk * 2 * CH:
                                 cs_off[k] + (q + 1) * nblk * 2 * CH]
                        .unsqueeze(0),
                        in_=csb[32 * q: 32 * q + 1, :])

                dro = (d_out[out_off[k]: out_off[k] + 384 * 3 * w]
                       .rearrange("(p a n) -> p a n", p=P, a=3))
                nc.sync.dma_start(
                    out=dro,
                    in_=row[:, :9 * w].rearrange("p (a n) -> p a n", a=3))
    nc.compile()
    return nc


def _pack(coords, num_atoms, slots):
    """Per-core input arrays for the SPMD program."""
    B = coords.shape[0]
    N = coords.shape[1] // 3
    widths = [s[0] for s in slots]
    K = len(slots)
    AUXW = P * K + 3 * K
    rh_off, cf_off, out_off, cs_off, rh_len, cf_len, out_len, cs_len = (
        _offsets(widths))
    c3 = coords.reshape(B, N, 3)
    pidx = np.arange(P)

    in_maps = []
    for _ in range(N_CORES):
        in_maps.append({
            "rhs": np.zeros((5, rh_len), np.float32),
            "cfp": np.zeros((1, cf_len), np.float32),
            "aux": np.zeros((P, AUXW), np.float32),
        })

    placement = []  # (core, k, b, t, wt)
    for k, (w, chunk) in enumerate(slots):
        for core, (wt, b, t) in enumerate(chunk):
            placement.append((core, k, b, t, wt))
            m = in_maps[core]
            na = int(num_atoms[b])
            j0 = t * P
            cf = c3[b, j0: j0 + w].astype(np.float64)   # [<=w, 3] local cols
            nw = cf.shape[0]
            ct = c3[b, j0: j0 + P].astype(np.float64)   # [<=P, 3] own tile
            np_ = ct.shape[0]
            colmask = (j0 + np.arange(nw)) < na
            rowvalid = (j0 + pidx) < na
            # d2 rhs block: out = -(q_p + |cf|^2+eps+BIG*~m - 2 ct.cf)
            o = rh_off[k]
            rr = m["rhs"]
            rr[0, o: o + w] = -1.0
            rr[1, o: o + nw] = -((cf * cf).sum(1) + EPS + BIG * (~colmask))
            rr[1, o + nw: o + w] = -BIG
            for c in range(3):
                rr[2 + c, o: o + nw] = 2.0 * cf[:, c]
            # coordinate planes for the broadcast (+cf, c-major)
            for c in range(3):
                m["cfp"][0, cf_off[k] + c * w: cf_off[k] + c * w + nw] = (
                    cf[:, c])
            # lhsT block rows: q, ones, ct_x, ct_y, ct_z
            a0 = P * k
            q = np.full(P, BIG)
            q[:np_] = (ct * ct).sum(1) + BIG * (~rowvalid[:np_])
            m["aux"][0, a0: a0 + P] = q
            m["aux"][1, a0: a0 + P] = 1.0
            for c in range(3):
                m["aux"][2 + c, a0: a0 + np_] = ct[:, c]
                # -ct_c as the per-partition bias for the s~ planes
                m["aux"][:np_, P * K + 3 * k + c] = -ct[:, c]
    return in_maps, placement


_NC_CACHE = {}


def _get_program(widths):
    key = tuple(widths)
    if key not in _NC_CACHE:
        _NC_CACHE[key] = _build(list(widths))
    return _NC_CACHE[key]


def _reassemble(results, coords_shape, num_atoms, slots, placement):
    B, threeN = coords_shape[0], coords_shape[1]
    widths = [s[0] for s in slots]
    _, _, out_off, cs_off, _, _, _, _ = _offsets(widths)
    out = np.zeros((B, threeN, threeN), np.float32)
    pidx = np.arange(P)
    a3 = np.arange(3)
    # diagonal accumulators per example/atom
    diag = [np.zeros((threeN // 3, 3, 3), np.float64) for _ in range(B)]
    for (core, k, b, t, wt) in placement:
        w = widths[k]
        res = results[core]
        blk = res["out"][out_off[k]: out_off[k] + 384 * 3 * w]
        blk = blk.reshape(384, 3 * w)
        r = 384 * t
        out[b, r:r + 384, r:r + 3 * wt] = blk[:, :3 * wt]
        # mirror the off-tile part (blocks are symmetric: plain transpose)
        if wt > P:
            out[b, r + 384: r + 3 * wt, r:r + 384] = blk[:, 384: 3 * wt].T
        # dac: row sums over the triangle window
        dg6 = res["dg"][k]                              # [P, 6]
        diag[b][t * P: t * P + P] += dg6[:, SYM6]       # [P, 3, 3]
        # column sums for atoms beyond the own tile
        if wt > P:
            nblk = _nblk(w)
            cs = res["cs"][cs_off[k]: cs_off[k] + QPB * nblk * 2 * CH]
            # export rows hold chunks c = blk*CPG + seg*QPB + q at
            # [q, blk, seg, :]; transpose to chunk order (blk, seg, q)
            cs = (cs.reshape(QPB, nblk, 2, CH).transpose(1, 2, 0, 3)
                  .reshape(-1)[:9 * w])
            cs3 = cs.reshape(3, w, 3)                   # [a, j', c]
            contrib = cs3[:, P:wt, :].transpose(1, 0, 2)  # [q', a, c]
            diag[b][t * P + P: t * P + wt] += contrib
    q3 = 3 * np.arange(threeN // 3)
    rows = q3[:, None, None] + a3[None, :, None]
    cols = q3[:, None, None] + a3[None, None, :]
    for b in range(B):
        out[b, rows, cols] = -diag[b].astype(np.float32)
    return out


LAST_RUN = None  # BassKernelResults of the most recent kernel() call


def kernel(coords, num_atoms, _trace=False):
    global LAST_RUN
    coords = np.ascontiguousarray(np.asarray(coords, dtype=np.float32))
    na = np.asarray(num_atoms).astype(np.int64)
    slots = _plan(na)
    widths = [s[0] for s in slots]
    nc = _get_program(widths)
    in_maps, placement = _pack(coords, na, slots)
    LAST_RUN = run_bass_kernel_spmd(
        nc, in_maps, list(range(N_CORES)), trace=_trace,
        tmpdir=os.environ.get("TRACE_DIR") if _trace else None)
    return _reassemble(LAST_RUN.results, coords.shape, na, slots, placement)


# revision 18
# speedup vs baseline: 2.0922x; 1.1535x over previous
"""Trainium2 Bass kernel: batched pairwise Hessian blocks (Coords2Stress).

For each example b:  out[b, 3i+a, 3j+c] = -sep_a*sep_c/(|sep|^2+eps) for the
off-diagonal atom blocks (masked to the valid atom count), with the 3x3
diagonal blocks overwritten by the negative row-sums.

v2: block-upper-triangle + engine-balanced.

The 3x3 blocks are symmetric in (a,c) AND block(i,j) == block(j,i), so the
device only computes the block upper triangle: item (b,t) covers rows
[128t,128t+128) x cols [128t, max(na,128(t+1))) in item-local column coords
j' = j - 128t.  The host mirrors each rectangle (plain transpose) into the
lower triangle.  Per-slot engine assignment:
  * GpSimd broadcasts the column-coordinate planes cfp[1,3w] -> cb[128,3w].
  * ScalarE (Act) builds s~_c = cb_c - ct_c (Identity with per-partition
    bias), and casts the finished row tile to fp16 for the column sums.
  * TensorE computes the negated masked d2 via the |x|^2+|y|^2-2xy expansion
    (k=5 matmul; validity masks folded in as +BIG so the reciprocal
    vanishes), and column sums of the fp16 row tile (ones-lhsT matmuls,
    3 chunks stacked per PSUM bank at base partitions 0/32/64).
  * VectorE: reciprocal_approx_fast(r0 = 1/-d2), g~_a = s~_a * r0, and the
    9 products row[p,a,j,c] = g~_a * s~_c (6 unique carry dac accumulation).
Diagonal blocks: diag(q) = -(dac_q + sum of exported column sums from items
strictly above q's tile); the host adds the exported pieces and writes the
blocks.  Unwritten output stays zero.
"""

import os
import sys

import numpy as np

for _p in ("/opt/trn_rl_repo", "/root/.axon_site/_ro/trn_rl_repo"):
    if os.path.isdir(_p) and _p not in sys.path:
        sys.path.insert(0, _p)

import concourse.bass as bass
import concourse.bacc as bacc
import concourse.tile as tile
from concourse import mybir
from concourse.bass_utils import run_bass_kernel_spmd

N_CORES = 8
P = 128  # atoms per work item == SBUF partitions
CH = 512  # matmul free-dim chunk (one PSUM bank of fp32)
QPB = 3  # colsum chunks stacked per PSUM bank (base partitions 0/32/64)
EPS = 1e-5
BIG = 1e30
F32 = mybir.dt.float32
F16 = mybir.dt.float16
OP = mybir.AluOpType
UNIQ = [(0, 0), (0, 1), (0, 2), (1, 1), (1, 2), (2, 2)]
MIRROR = [(1, 0), (2, 0), (2, 1)]
SYM6 = np.array([[0, 1, 2], [1, 3, 4], [2, 4, 5]])


def _plan(num_atoms):
    """Triangle work items -> slots.  Item (b,t) covers local columns
    [0, wt) with wt = max(na, 128(t+1)) - 128t.  Returns
    [(slot_width, [(wt, b, t), ...])] with slots of <= N_CORES items."""
    items = []
    for b, na in enumerate(num_atoms):
        na = int(na)
        if na <= 0:
            continue
        nt = -(-na // P)
        for t in range(nt):
            items.append((max(na, P * (t + 1)) - P * t, b, t))
    items.sort(key=lambda x: (-x[0], x[1], x[2]))
    nslot = max(1, -(-len(items) // N_CORES))
    slots = []
    for k in range(nslot):
        chunk = items[k * N_CORES:(k + 1) * N_CORES]
        slots.append((chunk[0][0], chunk))
    return slots


CPG = 6  # colsum chunks per PSUM group tile ([P, 2*CH] = 2 banks, 3 quadrants)


def _ncs(w):
    """Number of colsum chunks for slot width w."""
    return -(-9 * w // CH)


def _nblk(w):
    """Number of colsum PSUM group tiles (CPG chunks each)."""
    return -(-_ncs(w) // CPG)


def _offsets(widths):
    rh, cf, oo, cs = [], [], [], []
    a = b = c = d = 0
    for w in widths:
        rh.append(a)
        cf.append(b)
        oo.append(c)
        cs.append(d)
        a += w
        b += 3 * w
        c += 384 * 3 * w
        d += QPB * _nblk(w) * 2 * CH
    return rh, cf, oo, cs, a, b, c, d


def _chunks(w):
    return [(h, min(h + CH, w)) for h in range(0, w, CH)]


def _build(widths):
    """Emit + compile the SPMD program for the given per-slot widths."""
    K = len(widths)
    Wmax = max(widths)
    rh_off, cf_off, out_off, cs_off, rh_len, cf_len, out_len, cs_len = (
        _offsets(widths))
    AUXW = P * K + 3 * K

    nc = bacc.Bacc("TRN2", target_bir_lowering=False, debug=False)
    d_rhs = nc.dram_tensor("rhs", [5, rh_len], F32, kind="ExternalInput").ap()
    d_cfp = nc.dram_tensor("cfp", [1, cf_len], F32, kind="ExternalInput").ap()
    d_aux = nc.dram_tensor("aux", [P, AUXW], F32, kind="ExternalInput").ap()
    d_out = nc.dram_tensor("out", [out_len], F32, kind="ExternalOutput").ap()
    d_dg = nc.dram_tensor("dg", [K, P, 6], F32, kind="ExternalOutput").ap()
    d_cs = nc.dram_tensor("cs", [cs_len], F32, kind="ExternalOutput").ap()

    with tile.TileContext(nc) as tc:
        with (
            tc.tile_pool(name="const", bufs=1) as constp,
            tc.tile_pool(name="rhsp", bufs=2) as rhsp,
            tc.tile_pool(name="cfpp", bufs=1) as cfpp,
            tc.tile_pool(name="sp", bufs=2) as sp,
            tc.tile_pool(name="rp", bufs=2) as rp,
            tc.tile_pool(name="gp", bufs=2) as gp,
            tc.tile_pool(name="dac", bufs=2) as dacp,
            tc.tile_pool(name="row", bufs=2) as rowp,
            tc.tile_pool(name="rbp", bufs=2) as rbp,
            tc.tile_pool(name="csp", bufs=2) as csbp,
            tc.psum_pool(name="d2p", bufs=2) as d2pp,
            tc.psum_pool(name="csm", bufs=2) as csmp,
        ):
            aux = constp.tile([P, AUXW], F32)
            nc.scalar.dma_start(out=aux[:], in_=d_aux)
            onesh = constp.tile([P, 1], F16)
            nc.vector.memset(onesh[:], 1.0)

            for k, w in enumerate(widths):
                lhsT = aux[0:5, P * k: P * k + P]
                nblk = _nblk(w)

                rhs = rhsp.tile([5, Wmax], F32, tag="rhs")
                nc.scalar.dma_start(
                    out=rhs[:, :w], in_=d_rhs[:, rh_off[k]: rh_off[k] + w])
                cfp = cfpp.tile([1, 3 * Wmax], F32, tag="cfp")
                nc.scalar.dma_start(
                    out=cfp[:, :3 * w],
                    in_=d_cfp[:, cf_off[k]: cf_off[k] + 3 * w])

                # s~_c = cf_c - ct_c: gpsimd broadcasts the cf plane into the
                # s tile and adds the per-partition bias -ct_c in place
                s_pl = []
                for c in range(3):
                    s_c = sp.tile([P, Wmax], F32, tag=f"s{c}")
                    nc.gpsimd.partition_broadcast(
                        s_c[:, :w], cfp[:1, c * w: c * w + w])
                    nc.scalar.add(
                        s_c[:, :w], s_c[:, :w],
                        aux[:, P * K + 3 * k + c: P * K + 3 * k + c + 1])
                    s_pl.append(s_c)

                # negated masked d2 via TensorE (stays in PSUM for the recip)
                d2 = d2pp.tile([P, Wmax], F32, tag="d2")
                for (h0, h1) in _chunks(w):
                    nc.tensor.matmul(
                        d2[:, h0:h1], lhsT, rhs[:, h0:h1])

                # r0 = 1/(-d2)  (negative; masks make it ~0 where invalid)
                r0 = rp.tile([P, Wmax], F32, tag="r0")
                nc.vector.reciprocal_approx_fast(r0[:, :w], d2[:, :w])

                # g~_a = s~_a * r0
                g_pl = []
                for a in range(3):
                    g_a = gp.tile([P, Wmax], F32, tag=f"g{a}")
                    nc.vector.tensor_tensor(g_a[:, :w], s_pl[a][:, :w],
                                            r0[:, :w], OP.mult)
                    g_pl.append(g_a)

                # row[p, a, j, c] = g~_a * s~_c = -sep_a*sep_c*m/d2
                row = rowp.tile([P, 9 * Wmax], F32, tag="row")
                row4 = row[:, :9 * w].rearrange("p (a j c) -> p a j c",
                                                a=3, c=3)
                dac = dacp.tile([P, 8], F32, tag="dac")
                for i, (a, c) in enumerate(UNIQ):
                    nc.vector.scalar_tensor_tensor(
                        row4[:, a, :, c], s_pl[c][:, :w], 0.0, g_pl[a][:, :w],
                        OP.bypass, OP.mult,
                        accum_out=dac[:, i: i + 1])
                for (a, c) in MIRROR:
                    nc.vector.scalar_tensor_tensor(
                        row4[:, a, :, c], s_pl[c][:, :w], 0.0, g_pl[a][:, :w],
                        OP.bypass, OP.mult)
                nc.scalar.dma_start(out=d_dg[k], in_=dac[:, 0:6])

                # column sums: fp16 cast (Act) + ones-matmuls; CPG chunks per
                # PSUM group tile ([P, 2CH], quadrants 0/32/64 x 2 segments),
                # one Act drain per group, 3 export DMAs per slot
                csb = csbp.tile([P, nblk * 2 * CH], F32, tag="csb")
                for blk in range(nblk):
                    c0 = blk * CPG * CH
                    c1 = min(c0 + CPG * CH, 9 * w)
                    rb = rbp.tile([P, CPG * CH], F16, tag="rb")
                    nc.scalar.copy(rb[:, :c1 - c0], row[:, c0:c1])
                    ps = csmp.tile([P, 2 * CH], F32, tag="cs")
                    for j in range(CPG):
                        q0 = j * CH
                        q1 = min(q0 + CH, c1 - c0)
                        if q0 >= c1 - c0:
                            break
                        q, seg = j % QPB, j // QPB
                        nc.tensor.matmul(
                            ps[32 * q: 32 * q + 1,
                               seg * CH: seg * CH + q1 - q0], onesh[:],
                            rb[:, q0:q1])
                    nc.scalar.copy(csb[:, blk * 2 * CH:(blk + 1) * 2 * CH],
                                   ps[:])
                for q in range(QPB):
                    # row 32q of csb holds chunks with (c % CPG) % QPB == q
                    nc.gpsimd.dma_start(
                        out=d_cs[cs_off[k] + q * nblk * 2 * CH:
                                 cs_off[k] + (q + 1) * nblk * 2 * CH]
                        .unsqueeze(0),
                        in_=csb[32 * q: 32 * q + 1, :])

                dro = (d_out[out_off[k]: out_off[k] + 384 * 3 * w]
                       .rearrange("(p a n) -> p a n", p=P, a=3))
                nc.sync.dma_start(
                    out=dro,
                    in_=row[:, :9 * w].rearrange("p (a n) -> p a n", a=3))
    nc.compile()
    return nc


def _pack(coords, num_atoms, slots):
    """Per-core input arrays for the SPMD program."""
    B = coords.shape[0]
    N = coords.shape[1] // 3
    widths = [s[0] for s in slots]
    K = len(slots)
    AUXW = P * K + 3 * K
    rh_off, cf_off, out_off, cs_off, rh_len, cf_len, out_len, cs_len = (
        _offsets(widths))
    c3 = coords.reshape(B, N, 3)
    pidx = np.arange(P)

    in_maps = []
    for _ in range(N_CORES):
        in_maps.append({
            "rhs": np.zeros((5, rh_len), np.float32),
            "cfp": np.zeros((1, cf_len), np.float32),
            "aux": np.zeros((P, AUXW), np.float32),
        })

    placement = []  # (core, k, b, t, wt)
    for k, (w, chunk) in enumerate(slots):
        for core, (wt, b, t) in enumerate(chunk):
            placement.append((core, k, b, t, wt))
            m = in_maps[core]
            na = int(num_atoms[b])
            j0 = t * P
            cf = c3[b, j0: j0 + w].astype(np.float64)   # [<=w, 3] local cols
            nw = cf.shape[0]
            ct = c3[b, j0: j0 + P].astype(np.float64)   # [<=P, 3] own tile
            np_ = ct.shape[0]
            colmask = (j0 + np.arange(nw)) < na
            rowvalid = (j0 + pidx) < na
            # d2 rhs block: out = -(q_p + |cf|^2+eps+BIG*~m - 2 ct.cf)
            o = rh_off[k]
            rr = m["rhs"]
            rr[0, o: o + w] = -1.0
            rr[1, o: o + nw] = -((cf * cf).sum(1) + EPS + BIG * (~colmask))
            rr[1, o + nw: o + w] = -BIG
            for c in range(3):
                rr[2 + c, o: o + nw] = 2.0 * cf[:, c]
            # coordinate planes for the broadcast (+cf, c-major)
            for c in range(3):
                m["cfp"][0, cf_off[k] + c * w: cf_off[k] + c * w + nw] = (
                    cf[:, c])
            # lhsT block rows: q, ones, ct_x, ct_y, ct_z
            a0 = P * k
            q = np.full(P, BIG)
            q[:np_] = (ct * ct).sum(1) + BIG * (~rowvalid[:np_])
            m["aux"][0, a0: a0 + P] = q
            m["aux"][1, a0: a0 + P] = 1.0
            for c in range(3):
                m["aux"][2 + c, a0: a0 + np_] = ct[:, c]
                # -ct_c as the per-partition bias for the s~ planes
                m["aux"][:np_, P * K + 3 * k + c] = -ct[:, c]
    return in_maps, placement


_NC_CACHE = {}


def _get_program(widths):
    key = tuple(widths)
    if key not in _NC_CACHE:
        _NC_CACHE[key] = _build(list(widths))
    return _NC_CACHE[key]


def _reassemble(results, coords_shape, num_atoms, slots, placement):
    B, threeN = coords_shape[0], coords_shape[1]
    widths = [s[0] for s in slots]
    _, _, out_off, cs_off, _, _, _, _ = _offsets(widths)
    out = np.zeros((B, threeN, threeN), np.float32)
    pidx = np.arange(P)
    a3 = np.arange(3)
    # diagonal accumulators per example/atom
    diag = [np.zeros((threeN // 3, 3, 3), np.float64) for _ in range(B)]
    for (core, k, b, t, wt) in placement:
        w = widths[k]
        res = results[core]
        blk = res["out"][out_off[k]: out_off[k] + 384 * 3 * w]
        blk = blk.reshape(384, 3 * w)
        r = 384 * t
        out[b, r:r + 384, r:r + 3 * wt] = blk[:, :3 * wt]
        # mirror the off-tile part (blocks are symmetric: plain transpose)
        if wt > P:
            out[b, r + 384: r + 3 * wt, r:r + 384] = blk[:, 384: 3 * wt].T
        # dac: row sums over the triangle window
        dg6 = res["dg"][k]                              # [P, 6]
        diag[b][t * P: t * P + P] += dg6[:, SYM6]       # [P, 3, 3]
        # column sums for atoms beyond the own tile
        if wt > P:
            nblk = _nblk(w)
            cs = res["cs"][cs_off[k]: cs_off[k] + QPB * nblk * 2 * CH]
            # export rows hold chunks c = blk*CPG + seg*QPB + q at
            # [q, blk, seg, :]; transpose to chunk order (blk, seg, q)
            cs = (cs.reshape(QPB, nblk, 2, CH).transpose(1, 2, 0, 3)
                  .reshape(-1)[:9 * w])
            cs3 = cs.reshape(3, w, 3)                   # [a, j', c]
            contrib = cs3[:, P:wt, :].transpose(1, 0, 2)  # [q', a, c]
            diag[b][t * P + P: t * P + wt] += contrib
    q3 = 3 * np.arange(threeN // 3)
    rows = q3[:, None, None] + a3[None, :, None]
    cols = q3[:, None, None] + a3[None, None, :]
    for b in range(B):
        out[b, rows, cols] = -diag[b].astype(np.float32)
    return out


LAST_RUN = None  # BassKernelResults of the most recent kernel() call


def kernel(coords, num_atoms, _trace=False):
    global LAST_RUN
    coords = np.ascontiguousarray(np.asarray(coords, dtype=np.float32))
    na = np.asarray(num_atoms).astype(np.int64)
    slots = _plan(na)
    widths = [s[0] for s in slots]
    nc = _get_program(widths)
    in_maps, placement = _pack(coords, na, slots)
    LAST_RUN = run_bass_kernel_spmd(
        nc, in_maps, list(range(N_CORES)), trace=_trace,
        tmpdir=os.environ.get("TRACE_DIR") if _trace else None)
    return _reassemble(LAST_RUN.results, coords.shape, na, slots, placement)


# revision 20
# speedup vs baseline: 2.1377x; 1.0218x over previous
"""Trainium2 Bass kernel: batched pairwise Hessian blocks (Coords2Stress).

For each example b:  out[b, 3i+a, 3j+c] = -sep_a*sep_c/(|sep|^2+eps) for the
off-diagonal atom blocks (masked to the valid atom count), with the 3x3
diagonal blocks overwritten by the negative row-sums.

v2: block-upper-triangle + engine-balanced.

The 3x3 blocks are symmetric in (a,c) AND block(i,j) == block(j,i), so the
device only computes the block upper triangle: item (b,t) covers rows
[128t,128t+128) x cols [128t, max(na,128(t+1))) in item-local column coords
j' = j - 128t.  The host mirrors each rectangle (plain transpose) into the
lower triangle.  Per-slot engine assignment:
  * GpSimd broadcasts the column-coordinate planes cfp[1,3w] -> cb[128,3w].
  * ScalarE (Act) builds s~_c = cb_c - ct_c (Identity with per-partition
    bias), and casts the finished row tile to fp16 for the column sums.
  * TensorE computes the negated masked d2 via the |x|^2+|y|^2-2xy expansion
    (k=5 matmul; validity masks folded in as +BIG so the reciprocal
    vanishes), and column sums of the fp16 row tile (ones-lhsT matmuls,
    3 chunks stacked per PSUM bank at base partitions 0/32/64).
  * VectorE: reciprocal_approx_fast(r0 = 1/-d2), g~_a = s~_a * r0, and the
    9 products row[p,a,j,c] = g~_a * s~_c (6 unique carry dac accumulation).
Diagonal blocks: diag(q) = -(dac_q + sum of exported column sums from items
strictly above q's tile); the host adds the exported pieces and writes the
blocks.  Unwritten output stays zero.
"""

import os
import sys

import numpy as np

for _p in ("/opt/trn_rl_repo", "/root/.axon_site/_ro/trn_rl_repo"):
    if os.path.isdir(_p) and _p not in sys.path:
        sys.path.insert(0, _p)

import concourse.bass as bass
import concourse.bacc as bacc
import concourse.tile as tile
from concourse import mybir
from concourse.bass_utils import run_bass_kernel_spmd

N_CORES = 8
P = 128  # atoms per work item == SBUF partitions
CH = 512  # matmul free-dim chunk (one PSUM bank of fp32)
QPB = 3  # colsum chunks stacked per PSUM bank (base partitions 0/32/64)
EPS = 1e-5
BIG = 1e30
F32 = mybir.dt.float32
F16 = mybir.dt.float16
OP = mybir.AluOpType
UNIQ = [(0, 0), (0, 1), (0, 2), (1, 1), (1, 2), (2, 2)]
MIRROR = [(1, 0), (2, 0), (2, 1)]
SYM6 = np.array([[0, 1, 2], [1, 3, 4], [2, 4, 5]])


def _plan(num_atoms):
    """Triangle work items -> slots.  Item (b,t) covers local columns
    [0, wt) with wt = max(na, 128(t+1)) - 128t.  Returns
    [(slot_width, [(wt, b, t), ...])] with slots of <= N_CORES items."""
    items = []
    for b, na in enumerate(num_atoms):
        na = int(na)
        if na <= 0:
            continue
        nt = -(-na // P)
        for t in range(nt):
            items.append((max(na, P * (t + 1)) - P * t, b, t))
    items.sort(key=lambda x: (-x[0], x[1], x[2]))
    nslot = max(1, -(-len(items) // N_CORES))
    slots = []
    for k in range(nslot):
        chunk = items[k * N_CORES:(k + 1) * N_CORES]
        slots.append((chunk[0][0], chunk))
    return slots


CPG = 6  # colsum chunks per PSUM group tile ([P, 2*CH] = 2 banks, 3 quadrants)


def _ncs(w):
    """Number of colsum chunks for slot width w."""
    return -(-9 * w // CH)


def _nblk(w):
    """Number of colsum PSUM group tiles (CPG chunks each)."""
    return -(-_ncs(w) // CPG)


def _offsets(widths):
    rh, cf, oo, cs = [], [], [], []
    a = b = c = d = 0
    for w in widths:
        rh.append(a)
        cf.append(b)
        oo.append(c)
        cs.append(d)
        a += w
        b += 3 * w
        c += 384 * 3 * w
        d += QPB * _nblk(w) * 2 * CH
    return rh, cf, oo, cs, a, b, c, d


def _chunks(w):
    return [(h, min(h + CH, w)) for h in range(0, w, CH)]


def _build(widths):
    """Emit + compile the SPMD program for the given per-slot widths."""
    K = len(widths)
    Wmax = max(widths)
    rh_off, cf_off, out_off, cs_off, rh_len, cf_len, out_len, cs_len = (
        _offsets(widths))
    AUXW = P * K + 3 * K

    nc = bacc.Bacc("TRN2", target_bir_lowering=False, debug=False)
    d_rhs = nc.dram_tensor("rhs", [5, rh_len], F32, kind="ExternalInput").ap()
    d_cfp = nc.dram_tensor("cfp", [1, cf_len], F32, kind="ExternalInput").ap()
    d_aux = nc.dram_tensor("aux", [P, AUXW], F32, kind="ExternalInput").ap()
    d_out = nc.dram_tensor("out", [out_len], F32, kind="ExternalOutput").ap()
    d_dg = nc.dram_tensor("dg", [K, P, 6], F32, kind="ExternalOutput").ap()
    d_cs = nc.dram_tensor("cs", [cs_len], F32, kind="ExternalOutput").ap()

    with tile.TileContext(nc) as tc:
        with (
            tc.tile_pool(name="const", bufs=1) as constp,
            tc.tile_pool(name="rhsp", bufs=2) as rhsp,
            tc.tile_pool(name="cfpp", bufs=1) as cfpp,
            tc.tile_pool(name="sp", bufs=2) as sp,
            tc.tile_pool(name="rp", bufs=2) as rp,
            tc.tile_pool(name="gp", bufs=2) as gp,
            tc.tile_pool(name="dac", bufs=2) as dacp,
            tc.tile_pool(name="row", bufs=2) as rowp,
            tc.tile_pool(name="rbp", bufs=2) as rbp,
            tc.tile_pool(name="csp", bufs=2) as csbp,
            tc.psum_pool(name="d2p", bufs=2) as d2pp,
            tc.psum_pool(name="csm", bufs=2) as csmp,
        ):
            aux = constp.tile([P, AUXW], F32)
            nc.scalar.dma_start(out=aux[:], in_=d_aux)
            onesh = constp.tile([P, 1], F16)
            nc.vector.memset(onesh[:], 1.0)

            for k, w in enumerate(widths):
                lhsT = aux[0:5, P * k: P * k + P]
                nblk = _nblk(w)

                rhs = rhsp.tile([5, Wmax], F32, tag="rhs")
                nc.scalar.dma_start(
                    out=rhs[:, :w], in_=d_rhs[:, rh_off[k]: rh_off[k] + w])
                cfp = cfpp.tile([1, 3 * Wmax], F32, tag="cfp")
                nc.scalar.dma_start(
                    out=cfp[:, :3 * w],
                    in_=d_cfp[:, cf_off[k]: cf_off[k] + 3 * w])

                # cb_c = broadcast cf plane; s~_c = cb_c - ct_c is never
                # materialized -- every consuming STT op fuses the bias via
                # its scalar slot: (cb_c add -ct_c) mult <other>
                cb_pl = []
                negct = []
                for c in range(3):
                    cb_c = sp.tile([P, Wmax], F32, tag=f"s{c}")
                    nc.gpsimd.partition_broadcast(
                        cb_c[:, :w], cfp[:1, c * w: c * w + w])
                    cb_pl.append(cb_c)
                    negct.append(
                        aux[:, P * K + 3 * k + c: P * K + 3 * k + c + 1])

                # negated masked d2 via TensorE (stays in PSUM for the recip)
                d2 = d2pp.tile([P, Wmax], F32, tag="d2")
                for (h0, h1) in _chunks(w):
                    nc.tensor.matmul(
                        d2[:, h0:h1], lhsT, rhs[:, h0:h1])

                # r0 = 1/(-d2)  (negative; masks make it ~0 where invalid)
                r0 = rp.tile([P, Wmax], F32, tag="r0")
                nc.vector.reciprocal_approx_fast(r0[:, :w], d2[:, :w])

                # g~_a = (cb_a - ct_a) * r0
                g_pl = []
                for a in range(3):
                    g_a = gp.tile([P, Wmax], F32, tag=f"g{a}")
                    nc.vector.scalar_tensor_tensor(
                        g_a[:, :w], cb_pl[a][:, :w], negct[a], r0[:, :w],
                        OP.add, OP.mult)
                    g_pl.append(g_a)

                # row[p, a, j, c] = (cb_c - ct_c) * g~_a = -sep_a*sep_c*m/d2
                row = rowp.tile([P, 9 * Wmax], F32, tag="row")
                row4 = row[:, :9 * w].rearrange("p (a j c) -> p a j c",
                                                a=3, c=3)
                dac = dacp.tile([P, 8], F32, tag="dac")
                for i, (a, c) in enumerate(UNIQ):
                    nc.vector.scalar_tensor_tensor(
                        row4[:, a, :, c], cb_pl[c][:, :w], negct[c],
                        g_pl[a][:, :w], OP.add, OP.mult,
                        accum_out=dac[:, i: i + 1])
                for (a, c) in MIRROR:
                    nc.vector.scalar_tensor_tensor(
                        row4[:, a, :, c], cb_pl[c][:, :w], negct[c],
                        g_pl[a][:, :w], OP.add, OP.mult)
                nc.scalar.dma_start(out=d_dg[k], in_=dac[:, 0:6])

                # column sums: fp16 cast (Act) + ones-matmuls; CPG chunks per
                # PSUM group tile ([P, 2CH], quadrants 0/32/64 x 2 segments),
                # one Act drain per group, 3 export DMAs per slot
                csb = csbp.tile([P, nblk * 2 * CH], F32, tag="csb")
                for blk in range(nblk):
                    c0 = blk * CPG * CH
                    c1 = min(c0 + CPG * CH, 9 * w)
                    rb = rbp.tile([P, CPG * CH], F16, tag="rb")
                    nc.scalar.copy(rb[:, :c1 - c0], row[:, c0:c1])
                    ps = csmp.tile([P, 2 * CH], F32, tag="cs")
                    for j in range(CPG):
                        q0 = j * CH
                        q1 = min(q0 + CH, c1 - c0)
                        if q0 >= c1 - c0:
                            break
                        q, seg = j % QPB, j // QPB
                        nc.tensor.matmul(
                            ps[32 * q: 32 * q + 1,
                               seg * CH: seg * CH + q1 - q0], onesh[:],
                            rb[:, q0:q1])
                    nc.scalar.copy(csb[:, blk * 2 * CH:(blk + 1) * 2 * CH],
                                   ps[:])
                for q in range(QPB):
                    # row 32q of csb holds chunks with (c % CPG) % QPB == q
                    nc.gpsimd.dma_start(
                        out=d_cs[cs_off[k] + q * nblk * 2 * CH:
                                 cs_off[k] + (q + 1) * nblk * 2 * CH]
                        .unsqueeze(0),
                        in_=csb[32 * q: 32 * q + 1, :])

                dro = (d_out[out_off[k]: out_off[k] + 384 * 3 * w]
                       .rearrange("(p a n) -> p a n", p=P, a=3))
                nc.sync.dma_start(
                    out=dro,
                    in_=row[:, :9 * w].rearrange("p (a n) -> p a n", a=3))
    nc.compile()
    return nc


def _pack(coords, num_atoms, slots):
    """Per-core input arrays for the SPMD program."""
    B = coords.shape[0]
    N = coords.shape[1] // 3
    widths = [s[0] for s in slots]
    K = len(slots)
    AUXW = P * K + 3 * K
    rh_off, cf_off, out_off, cs_off, rh_len, cf_len, out_len, cs_len = (
        _offsets(widths))
    c3 = coords.reshape(B, N, 3)
    pidx = np.arange(P)

    in_maps = []
    for _ in range(N_CORES):
        in_maps.append({
            "rhs": np.zeros((5, rh_len), np.float32),
            "cfp": np.zeros((1, cf_len), np.float32),
            "aux": np.zeros((P, AUXW), np.float32),
        })

    placement = []  # (core, k, b, t, wt)
    for k, (w, chunk) in enumerate(slots):
        for core, (wt, b, t) in enumerate(chunk):
            placement.append((core, k, b, t, wt))
            m = in_maps[core]
            na = int(num_atoms[b])
            j0 = t * P
            cf = c3[b, j0: j0 + w].astype(np.float64)   # [<=w, 3] local cols
            nw = cf.shape[0]
            ct = c3[b, j0: j0 + P].astype(np.float64)   # [<=P, 3] own tile
            np_ = ct.shape[0]
            colmask = (j0 + np.arange(nw)) < na
            rowvalid = (j0 + pidx) < na
            # d2 rhs block: out = -(q_p + |cf|^2+eps+BIG*~m - 2 ct.cf)
            o = rh_off[k]
            rr = m["rhs"]
            rr[0, o: o + w] = -1.0
            rr[1, o: o + nw] = -((cf * cf).sum(1) + EPS + BIG * (~colmask))
            rr[1, o + nw: o + w] = -BIG
            for c in range(3):
                rr[2 + c, o: o + nw] = 2.0 * cf[:, c]
            # coordinate planes for the broadcast (+cf, c-major)
            for c in range(3):
                m["cfp"][0, cf_off[k] + c * w: cf_off[k] + c * w + nw] = (
                    cf[:, c])
            # lhsT block rows: q, ones, ct_x, ct_y, ct_z
            a0 = P * k
            q = np.full(P, BIG)
            q[:np_] = (ct * ct).sum(1) + BIG * (~rowvalid[:np_])
            m["aux"][0, a0: a0 + P] = q
            m["aux"][1, a0: a0 + P] = 1.0
            for c in range(3):
                m["aux"][2 + c, a0: a0 + np_] = ct[:, c]
                # -ct_c as the per-partition bias for the s~ planes
                m["aux"][:np_, P * K + 3 * k + c] = -ct[:, c]
    return in_maps, placement


_NC_CACHE = {}


def _get_program(widths):
    key = tuple(widths)
    if key not in _NC_CACHE:
        _NC_CACHE[key] = _build(list(widths))
    return _NC_CACHE[key]


def _reassemble(results, coords_shape, num_atoms, slots, placement):
    B, threeN = coords_shape[0], coords_shape[1]
    widths = [s[0] for s in slots]
    _, _, out_off, cs_off, _, _, _, _ = _offsets(widths)
    out = np.zeros((B, threeN, threeN), np.float32)
    pidx = np.arange(P)
    a3 = np.arange(3)
    # diagonal accumulators per example/atom
    diag = [np.zeros((threeN // 3, 3, 3), np.float64) for _ in range(B)]
    for (core, k, b, t, wt) in placement:
        w = widths[k]
        res = results[core]
        blk = res["out"][out_off[k]: out_off[k] + 384 * 3 * w]
        blk = blk.reshape(384, 3 * w)
        r = 384 * t
        out[b, r:r + 384, r:r + 3 * wt] = blk[:, :3 * wt]
        # mirror the off-tile part (blocks are symmetric: plain transpose)
        if wt > P:
            out[b, r + 384: r + 3 * wt, r:r + 384] = blk[:, 384: 3 * wt].T
        # dac: row sums over the triangle window
        dg6 = res["dg"][k]                              # [P, 6]
        diag[b][t * P: t * P + P] += dg6[:, SYM6]       # [P, 3, 3]
        # column sums for atoms beyond the own tile
        if wt > P:
            nblk = _nblk(w)
            cs = res["cs"][cs_off[k]: cs_off[k] + QPB * nblk * 2 * CH]
            # export rows hold chunks c = blk*CPG + seg*QPB + q at
            # [q, blk, seg, :]; transpose to chunk order (blk, seg, q)
            cs = (cs.reshape(QPB, nblk, 2, CH).transpose(1, 2, 0, 3)
                  .reshape(-1)[:9 * w])
            cs3 = cs.reshape(3, w, 3)                   # [a, j', c]
            contrib = cs3[:, P:wt, :].transpose(1, 0, 2)  # [q', a, c]
            diag[b][t * P + P: t * P + wt] += contrib
    q3 = 3 * np.arange(threeN // 3)
    rows = q3[:, None, None] + a3[None, :, None]
    cols = q3[:, None, None] + a3[None, None, :]
    for b in range(B):
        out[b, rows, cols] = -diag[b].astype(np.float32)
    return out


LAST_RUN = None  # BassKernelResults of the most recent kernel() call


def kernel(coords, num_atoms, _trace=False):
    global LAST_RUN
    coords = np.ascontiguousarray(np.asarray(coords, dtype=np.float32))
    na = np.asarray(num_atoms).astype(np.int64)
    slots = _plan(na)
    widths = [s[0] for s in slots]
    nc = _get_program(widths)
    in_maps, placement = _pack(coords, na, slots)
    LAST_RUN = run_bass_kernel_spmd(
        nc, in_maps, list(range(N_CORES)), trace=_trace,
        tmpdir=os.environ.get("TRACE_DIR") if _trace else None)
    return _reassemble(LAST_RUN.results, coords.shape, na, slots, placement)


# revision 24
# speedup vs baseline: 2.4560x; 1.1489x over previous
"""Trainium2 Bass kernel: batched pairwise Hessian blocks (Coords2Stress).

For each example b:  out[b, 3i+a, 3j+c] = -sep_a*sep_c/(|sep|^2+eps) for the
off-diagonal atom blocks (masked to the valid atom count), with the 3x3
diagonal blocks overwritten by the negative row-sums.

v2: block-upper-triangle + engine-balanced.

The 3x3 blocks are symmetric in (a,c) AND block(i,j) == block(j,i), so the
device only computes the block upper triangle: item (b,t) covers rows
[128t,128t+128) x cols [128t, max(na,128(t+1))) in item-local column coords
j' = j - 128t.  The host mirrors each rectangle (plain transpose) into the
lower triangle.  Per-slot engine assignment:
  * GpSimd broadcasts the column-coordinate planes cfp[1,3w] -> cb[128,3w].
  * ScalarE (Act) builds s~_c = cb_c - ct_c (Identity with per-partition
    bias), and casts the finished row tile to fp16 for the column sums.
  * TensorE computes the negated masked d2 via the |x|^2+|y|^2-2xy expansion
    (k=5 matmul; validity masks folded in as +BIG so the reciprocal
    vanishes), and column sums of the fp16 row tile (ones-lhsT matmuls,
    3 chunks stacked per PSUM bank at base partitions 0/32/64).
  * VectorE: reciprocal_approx_fast(r0 = 1/-d2), g~_a = s~_a * r0, and the
    9 products row[p,a,j,c] = g~_a * s~_c (6 unique carry dac accumulation).
Diagonal blocks: diag(q) = -(dac_q + sum of exported column sums from items
strictly above q's tile); the host adds the exported pieces and writes the
blocks.  Unwritten output stays zero.
"""

import os
import sys

import numpy as np

for _p in ("/opt/trn_rl_repo", "/root/.axon_site/_ro/trn_rl_repo"):
    if os.path.isdir(_p) and _p not in sys.path:
        sys.path.insert(0, _p)

import concourse.bass as bass
import concourse.bacc as bacc
import concourse.tile as tile
from concourse import mybir
from concourse.bass_utils import run_bass_kernel_spmd

N_CORES = 8
P = 128  # atoms per work item == SBUF partitions
CH = 512  # matmul free-dim chunk (one PSUM bank of fp32)
QPB = 3  # colsum chunks stacked per PSUM bank (base partitions 0/32/64)
EPS = 1e-5
BIG = 1e30
F32 = mybir.dt.float32
F16 = mybir.dt.float16
OP = mybir.AluOpType
UNIQ = [(0, 0), (0, 1), (0, 2), (1, 1), (1, 2), (2, 2)]
MIRROR = [(1, 0), (2, 0), (2, 1)]
SYM6 = np.array([[0, 1, 2], [1, 3, 4], [2, 4, 5]])


def _plan(num_atoms):
    """Triangle work items -> slots.  Item (b,t) covers local columns
    [0, wt) with wt = max(na, 128(t+1)) - 128t.  Returns
    [(slot_width, [(wt, b, t), ...])] with slots of <= N_CORES items."""
    items = []
    for b, na in enumerate(num_atoms):
        na = int(na)
        if na <= 0:
            continue
        nt = -(-na // P)
        for t in range(nt):
            items.append((max(na, P * (t + 1)) - P * t, b, t))
    items.sort(key=lambda x: (-x[0], x[1], x[2]))
    nslot = max(1, -(-len(items) // N_CORES))
    slots = []
    for k in range(nslot):
        chunk = items[k * N_CORES:(k + 1) * N_CORES]
        slots.append((chunk[0][0], chunk))
    # smallest slot first: cheap pipeline ramp, big slots overlap fully
    if len(slots) > 1:
        slots = [slots[-1]] + slots[:-1]
    return slots


CPG = 6  # colsum chunks per PSUM group tile ([P, 2*CH] = 2 banks, 3 quadrants)


def _ncs(w):
    """Number of colsum chunks for slot width w."""
    return -(-9 * w // CH)


def _nblk(w):
    """Number of colsum PSUM group tiles (CPG chunks each)."""
    return -(-_ncs(w) // CPG)


def _offsets(widths):
    rh, cf, oo, cs = [], [], [], []
    a = b = c = d = 0
    for w in widths:
        rh.append(a)
        cf.append(b)
        oo.append(c)
        cs.append(d)
        a += w
        b += 3 * w
        c += 384 * 3 * w
        d += QPB * _nblk(w) * 2 * CH
    return rh, cf, oo, cs, a, b, c, d


def _chunks(w):
    return [(h, min(h + CH, w)) for h in range(0, w, CH)]


def _build(widths):
    """Emit + compile the SPMD program for the given per-slot widths."""
    K = len(widths)
    Wmax = max(widths)
    rh_off, cf_off, out_off, cs_off, rh_len, cf_len, out_len, cs_len = (
        _offsets(widths))
    AUXW = P * K + 3 * K

    nc = bacc.Bacc("TRN2", target_bir_lowering=False, debug=False)
    d_rhs = nc.dram_tensor("rhs", [5, rh_len], F32, kind="ExternalInput").ap()
    d_cfp = nc.dram_tensor("cfp", [1, cf_len], F32, kind="ExternalInput").ap()
    d_aux = nc.dram_tensor("aux", [P, AUXW], F32, kind="ExternalInput").ap()
    d_out = nc.dram_tensor("out", [out_len], F32, kind="ExternalOutput").ap()
    d_dg = nc.dram_tensor("dg", [K, P, 6], F32, kind="ExternalOutput").ap()
    d_cs = nc.dram_tensor("cs", [cs_len], F32, kind="ExternalOutput").ap()

    with tile.TileContext(nc) as tc:
        with (
            tc.tile_pool(name="const", bufs=1) as constp,
            tc.tile_pool(name="rhsp", bufs=2) as rhsp,
            tc.tile_pool(name="cfpp", bufs=1) as cfpp,
            tc.tile_pool(name="sp", bufs=2) as sp,
            tc.tile_pool(name="rp", bufs=2) as rp,
            tc.tile_pool(name="gp", bufs=2) as gp,
            tc.tile_pool(name="dac", bufs=2) as dacp,
            tc.tile_pool(name="row", bufs=2) as rowp,
            tc.tile_pool(name="rbp", bufs=2) as rbp,
            tc.tile_pool(name="csp", bufs=2) as csbp,
            tc.psum_pool(name="d2p", bufs=2) as d2pp,
            tc.psum_pool(name="csm", bufs=2) as csmp,
        ):
            aux = constp.tile([P, AUXW], F32)
            nc.scalar.dma_start(out=aux[:], in_=d_aux)
            onesh = constp.tile([P, 1], F16)
            nc.vector.memset(onesh[:], 1.0)

            for k, w in enumerate(widths):
                lhsT = aux[0:5, P * k: P * k + P]
                nblk = _nblk(w)

                rhs = rhsp.tile([5, Wmax], F32, tag="rhs")
                nc.scalar.dma_start(
                    out=rhs[:, :w], in_=d_rhs[:, rh_off[k]: rh_off[k] + w])
                cfp = cfpp.tile([1, 3 * Wmax], F32, tag="cfp")
                nc.scalar.dma_start(
                    out=cfp[:, :3 * w],
                    in_=d_cfp[:, cf_off[k]: cf_off[k] + 3 * w])

                # cb_c = broadcast cf plane; s~_c = cb_c - ct_c is never
                # materialized -- every consuming STT op fuses the bias via
                # its scalar slot: (cb_c add -ct_c) mult <other>
                cb_pl = []
                negct = []
                for c in range(3):
                    cb_c = sp.tile([P, Wmax], F32, tag=f"s{c}")
                    nc.gpsimd.partition_broadcast(
                        cb_c[:, :w], cfp[:1, c * w: c * w + w])
                    cb_pl.append(cb_c)
                    negct.append(
                        aux[:, P * K + 3 * k + c: P * K + 3 * k + c + 1])

                # negated masked d2 via TensorE (stays in PSUM for the recip)
                d2 = d2pp.tile([P, Wmax], F32, tag="d2")
                for (h0, h1) in _chunks(w):
                    nc.tensor.matmul(
                        d2[:, h0:h1], lhsT, rhs[:, h0:h1])

                # r0 = 1/(-d2)  (negative; masks make it ~0 where invalid)
                r0 = rp.tile([P, Wmax], F32, tag="r0")
                nc.vector.reciprocal_approx_fast(r0[:, :w], d2[:, :w])

                # g~_a = (cb_a - ct_a) * r0
                g_pl = []
                for a in range(3):
                    g_a = gp.tile([P, Wmax], F32, tag=f"g{a}")
                    nc.vector.scalar_tensor_tensor(
                        g_a[:, :w], cb_pl[a][:, :w], negct[a], r0[:, :w],
                        OP.add, OP.mult)
                    g_pl.append(g_a)

                # row[p, (3a+c)*w + j] = (cb_c - ct_c) * g~_a: plane-major
                # (a c j) layout -- every write contiguous; the host permutes
                # to (a j c) during reassembly
                row = rowp.tile([P, 9 * Wmax], F32, tag="row")
                dac = dacp.tile([P, 8], F32, tag="dac")
                for i, (a, c) in enumerate(UNIQ):
                    nc.vector.scalar_tensor_tensor(
                        row[:, (3 * a + c) * w: (3 * a + c + 1) * w],
                        cb_pl[c][:, :w], negct[c],
                        g_pl[a][:, :w], OP.add, OP.mult,
                        accum_out=dac[:, i: i + 1])
                for (a, c) in MIRROR:
                    nc.vector.scalar_tensor_tensor(
                        row[:, (3 * a + c) * w: (3 * a + c + 1) * w],
                        cb_pl[c][:, :w], negct[c],
                        g_pl[a][:, :w], OP.add, OP.mult)
                nc.scalar.dma_start(out=d_dg[k], in_=dac[:, 0:6])

                # column sums: fp16 cast (Act) + ones-matmuls; CPG chunks per
                # PSUM group tile ([P, 2CH], quadrants 0/32/64 x 2 segments),
                # one Act drain per group, 3 export DMAs per slot
                csb = csbp.tile([P, nblk * 2 * CH], F32, tag="csb")
                for blk in range(nblk):
                    c0 = blk * CPG * CH
                    c1 = min(c0 + CPG * CH, 9 * w)
                    rb = rbp.tile([P, CPG * CH], F16, tag="rb")
                    nc.scalar.copy(rb[:, :c1 - c0], row[:, c0:c1])
                    ps = csmp.tile([P, 2 * CH], F32, tag="cs")
                    for j in range(CPG):
                        q0 = j * CH
                        q1 = min(q0 + CH, c1 - c0)
                        if q0 >= c1 - c0:
                            break
                        q, seg = j % QPB, j // QPB
                        nc.tensor.matmul(
                            ps[32 * q: 32 * q + 1,
                               seg * CH: seg * CH + q1 - q0], onesh[:],
                            rb[:, q0:q1])
                    nc.scalar.copy(csb[:, blk * 2 * CH:(blk + 1) * 2 * CH],
                                   ps[:])
                for q in range(QPB):
                    # row 32q of csb holds chunks with (c % CPG) % QPB == q
                    nc.gpsimd.dma_start(
                        out=d_cs[cs_off[k] + q * nblk * 2 * CH:
                                 cs_off[k] + (q + 1) * nblk * 2 * CH]
                        .unsqueeze(0),
                        in_=csb[32 * q: 32 * q + 1, :])

                dro = (d_out[out_off[k]: out_off[k] + 384 * 3 * w]
                       .rearrange("(p a n) -> p a n", p=P, a=3))
                nc.sync.dma_start(
                    out=dro,
                    in_=row[:, :9 * w].rearrange("p (a n) -> p a n", a=3))
    nc.compile()
    return nc


def _pack(coords, num_atoms, slots):
    """Per-core input arrays for the SPMD program."""
    B = coords.shape[0]
    N = coords.shape[1] // 3
    widths = [s[0] for s in slots]
    K = len(slots)
    AUXW = P * K + 3 * K
    rh_off, cf_off, out_off, cs_off, rh_len, cf_len, out_len, cs_len = (
        _offsets(widths))
    c3 = coords.reshape(B, N, 3)
    pidx = np.arange(P)

    in_maps = []
    for _ in range(N_CORES):
        in_maps.append({
            "rhs": np.zeros((5, rh_len), np.float32),
            "cfp": np.zeros((1, cf_len), np.float32),
            "aux": np.zeros((P, AUXW), np.float32),
        })

    placement = []  # (core, k, b, t, wt)
    for k, (w, chunk) in enumerate(slots):
        for core, (wt, b, t) in enumerate(chunk):
            placement.append((core, k, b, t, wt))
            m = in_maps[core]
            na = int(num_atoms[b])
            j0 = t * P
            cf = c3[b, j0: j0 + w].astype(np.float64)   # [<=w, 3] local cols
            nw = cf.shape[0]
            ct = c3[b, j0: j0 + P].astype(np.float64)   # [<=P, 3] own tile
            np_ = ct.shape[0]
            colmask = (j0 + np.arange(nw)) < na
            rowvalid = (j0 + pidx) < na
            # d2 rhs block: out = -(q_p + |cf|^2+eps+BIG*~m - 2 ct.cf)
            o = rh_off[k]
            rr = m["rhs"]
            rr[0, o: o + w] = -1.0
            rr[1, o: o + nw] = -((cf * cf).sum(1) + EPS + BIG * (~colmask))
            rr[1, o + nw: o + w] = -BIG
            for c in range(3):
                rr[2 + c, o: o + nw] = 2.0 * cf[:, c]
            # coordinate planes for the broadcast (+cf, c-major)
            for c in range(3):
                m["cfp"][0, cf_off[k] + c * w: cf_off[k] + c * w + nw] = (
                    cf[:, c])
            # lhsT block rows: q, ones, ct_x, ct_y, ct_z
            a0 = P * k
            q = np.full(P, BIG)
            q[:np_] = (ct * ct).sum(1) + BIG * (~rowvalid[:np_])
            m["aux"][0, a0: a0 + P] = q
            m["aux"][1, a0: a0 + P] = 1.0
            for c in range(3):
                m["aux"][2 + c, a0: a0 + np_] = ct[:, c]
                # -ct_c as the per-partition bias for the s~ planes
                m["aux"][:np_, P * K + 3 * k + c] = -ct[:, c]
    return in_maps, placement


_NC_CACHE = {}


def _get_program(widths):
    key = tuple(widths)
    if key not in _NC_CACHE:
        _NC_CACHE[key] = _build(list(widths))
    return _NC_CACHE[key]


def _reassemble(results, coords_shape, num_atoms, slots, placement):
    B, threeN = coords_shape[0], coords_shape[1]
    widths = [s[0] for s in slots]
    _, _, out_off, cs_off, _, _, _, _ = _offsets(widths)
    out = np.zeros((B, threeN, threeN), np.float32)
    pidx = np.arange(P)
    a3 = np.arange(3)
    # diagonal accumulators per example/atom
    diag = [np.zeros((threeN // 3, 3, 3), np.float64) for _ in range(B)]
    for (core, k, b, t, wt) in placement:
        w = widths[k]
        res = results[core]
        blk = res["out"][out_off[k]: out_off[k] + 384 * 3 * w]
        # device layout (p, a, c, j) -> (p, a, j, c) = [384, 3w]
        blk = (blk.reshape(P, 3, 3, w).transpose(0, 1, 3, 2)
               .reshape(384, 3 * w))
        r = 384 * t
        out[b, r:r + 384, r:r + 3 * wt] = blk[:, :3 * wt]
        # mirror the off-tile part (blocks are symmetric: plain transpose)
        if wt > P:
            out[b, r + 384: r + 3 * wt, r:r + 384] = blk[:, 384: 3 * wt].T
        # dac: row sums over the triangle window
        dg6 = res["dg"][k]                              # [P, 6]
        diag[b][t * P: t * P + P] += dg6[:, SYM6]       # [P, 3, 3]
        # column sums for atoms beyond the own tile
        if wt > P:
            nblk = _nblk(w)
            cs = res["cs"][cs_off[k]: cs_off[k] + QPB * nblk * 2 * CH]
            # export rows hold chunks c = blk*CPG + seg*QPB + q at
            # [q, blk, seg, :]; transpose to chunk order (blk, seg, q)
            cs = (cs.reshape(QPB, nblk, 2, CH).transpose(1, 2, 0, 3)
                  .reshape(-1)[:9 * w])
            cs3 = cs.reshape(3, 3, w)                   # [a, c, j']
            contrib = cs3[:, :, P:wt].transpose(2, 0, 1)  # [q', a, c]
            diag[b][t * P + P: t * P + wt] += contrib
    q3 = 3 * np.arange(threeN // 3)
    rows = q3[:, None, None] + a3[None, :, None]
    cols = q3[:, None, None] + a3[None, None, :]
    for b in range(B):
        out[b, rows, cols] = -diag[b].astype(np.float32)
    return out


LAST_RUN = None  # BassKernelResults of the most recent kernel() call


def kernel(coords, num_atoms, _trace=False):
    global LAST_RUN
    coords = np.ascontiguousarray(np.asarray(coords, dtype=np.float32))
    na = np.asarray(num_atoms).astype(np.int64)
    slots = _plan(na)
    widths = [s[0] for s in slots]
    nc = _get_program(widths)
    in_maps, placement = _pack(coords, na, slots)
    LAST_RUN = run_bass_kernel_spmd(
        nc, in_maps, list(range(N_CORES)), trace=_trace,
        tmpdir=os.environ.get("TRACE_DIR") if _trace else None)
    return _reassemble(LAST_RUN.results, coords.shape, na, slots, placement)
